# revision 25
# baseline (speedup 1.0000x reference)
"""Causal self-attention (B=4, T=2048, C=768, H=12) on 8 trn2 NeuronCores.

Sharding: core c -> batch b = c//2, head-half hh = c%2 (6 heads per core).
Each core computes, for its (b, 6 heads): qkv projection, causal attention,
and a partial output projection (its heads' rows of W_proj). The host sums
the two partial outputs per batch and adds b_proj.

All matmul operands are bf16 (fp32 accumulation in PSUM). Layouts keep the
PE contracting over partitions everywhere and softmax needs no transposes:
  - q^T [d, T] wide per pair; k^T stored as dense [128,128] tiles per
    (pair, k-block) so the S weight loads take the fast-weight-load path
  - S^T [tk, tq] blocks: lhsT = k tile, rhs = q^T chunk; two heads of a
    pair run concurrently via row groups (base partitions 0/64, K=64)
  - exp on ACT straight out of PSUM ([128, 1024] pair tiles, causal-skipped,
    diagonal strips merged into one 3D-AP activation)
  - PV: one dense [v_h | ones] (head A) / [ones | v_h] (head B) stationary
    per (t, head) -> a single full-width matmul per head per block computes
    both O^T and the softmax denominator replicated on the opposite 64
    partitions; normalize = reciprocal + partition-swap DMA + multiplies
    (multiplies on gpsimd, from an SBUF copy that frees PSUM early)
  - attention units are software-pipelined across unit boundaries (the next
    unit's first S blocks are emitted before the current unit's PV tail) so
    the scalar engine's exp stream never waits for the PV drain
  - proj is emitted per t-tile, spread into later units' slack; output DMA
    streams per tile
"""

import numpy as np

B, T, C = 4, 2048, 768
H = 12
D = C // H          # 64
HPC = 6             # heads per core
NP = 3              # head pairs per core
N_CORES = 8
TK = T // 128       # 16 t tiles
NCH = T // 512      # 4 q chunks
CT = C // 128       # 6 contraction tiles
NHEAD = 3           # S blocks emitted ahead at unit boundaries

_cache = {}


def _build(has_bias):
    import concourse.tile as tile
    from concourse import bacc, mybir

    dt = mybir.dt
    f32 = dt.float32
    bf16 = dt.bfloat16
    Exp = mybir.ActivationFunctionType.Exp

    nc = bacc.Bacc("TRN2", target_bir_lowering=False, debug=False,
                   num_devices=N_CORES)

    xT_ap = nc.dram_tensor("xT", [C, T], bf16, kind="ExternalInput").ap()
    wa_ap = nc.dram_tensor("wa", [C, 1152], bf16, kind="ExternalInput").ap()
    wp_ap = nc.dram_tensor("wp", [HPC * D, C], bf16, kind="ExternalInput").ap()
    tri_ap = nc.dram_tensor("tri", [128, 128], bf16, kind="ExternalInput").ap()
    if has_bias:
        ox_ap = nc.dram_tensor("ox", [1, T], bf16, kind="ExternalInput").ap()
        wb_ap = nc.dram_tensor("wb", [1, 1152], bf16, kind="ExternalInput").ap()
    out_ap = nc.dram_tensor("out", [T, C], f32, kind="ExternalOutput").ap()

    with tile.TileContext(nc) as tc:
        with tc.tile_pool(name="pers", bufs=1) as pers, \
             tc.tile_pool(name="pP", bufs=6) as pP, \
             tc.tile_pool(name="pst", bufs=2) as pst, \
             tc.tile_pool(name="pout", bufs=2) as pout, \
             tc.tile_pool(name="psA", bufs=2, space="PSUM") as psA, \
             tc.tile_pool(name="psOD", bufs=2, space="PSUM") as psOD:

            # ---- persistent SBUF tensors + staged input DMA ----
            # wa columns (host layout): q0 k0 q1 k1 q2 k2 (128 each) | v (384)
            xT = [pers.tile([128, T], bf16, tag=f"x{i}", name=f"x{i}")
                  for i in range(CT)]
            wa = [pers.tile([128, 1152], bf16, tag=f"w{i}", name=f"w{i}")
                  for i in range(CT)]
            # priority order: pair0 q/k weights interleaved with x's first
            # quarter, then the rest of x, v weights, pair1/2 q/k weights.
            for i in range(CT):
                nc.sync.dma_start(wa[i][:, 0:256],
                                  wa_ap[i * 128:(i + 1) * 128, 0:256])
                nc.sync.dma_start(xT[i][:, 0:512],
                                  xT_ap[i * 128:(i + 1) * 128, 0:512])
            for q in range(1, 4):
                for i in range(CT):
                    nc.sync.dma_start(xT[i][:, q * 512:(q + 1) * 512],
                                      xT_ap[i * 128:(i + 1) * 128,
                                            q * 512:(q + 1) * 512])
            for i in range(CT):
                nc.sync.dma_start(wa[i][:, 768:1152],
                                  wa_ap[i * 128:(i + 1) * 128, 768:1152])
            for i in range(CT):
                nc.sync.dma_start(wa[i][:, 256:768],
                                  wa_ap[i * 128:(i + 1) * 128, 256:768])
            wp = [pers.tile([128, C], bf16, tag=f"p{i}", name=f"wp{i}")
                  for i in range(NP)]
            for i in range(NP):
                nc.sync.dma_start(wp[i][:], wp_ap[i * 128:(i + 1) * 128, :])
            tri_b = pers.tile([128, 128], bf16, tag="trib")
            nc.sync.dma_start(tri_b[:], tri_ap)
            if has_bias:
                ox = pers.tile([1, T], bf16, tag="ox")
                nc.sync.dma_start(ox[:], ox_ap)
                wb = pers.tile([1, 1152], bf16, tag="wb")
                nc.sync.dma_start(wb[:], wb_ap)
            ones_f = pers.tile([128, 64], f32, tag="onesf")
            nc.vector.memset(ones_f[:], 1.0)
            ones_b = pers.tile([128, 64], bf16, tag="onesb")
            nc.vector.tensor_copy(ones_b[:], ones_f[:])

            # q^T wide per pair; k as dense [128,128] tiles per (pair, i)
            qw = [pers.tile([128, T], bf16, tag=f"q{p}", name=f"q{p}")
                  for p in range(NP)]
            kd = [[pers.tile([128, 128], bf16, tag=f"k{p}_{i}",
                             name=f"k{p}_{i}")
                   for i in range(TK)] for p in range(NP)]
            # [v_h | ones] (even h) / [ones | v_h] (odd h) stationaries
            vo = [[pers.tile([128, 128], bf16, tag=f"v{t}_{h}",
                             name=f"v{t}_{h}")
                   for h in range(HPC)] for t in range(TK)]
            for t in range(TK):
                for h in range(HPC):
                    half = 64 if h % 2 == 0 else 0
                    nc.sync.dma_start(vo[t][h][:, half:half + 64], ones_b[:])
            yt = [pers.tile([128, T], bf16, tag=f"y{p}", name=f"y{p}")
                  for p in range(NP)]

            # ---- qkv projection pieces (per 1024-col half of T) ----
            def emit_qk(p, cps=(0, 1)):
                with nc.named_scope(f"qk{p}"):
                    for qsel in range(2):          # 0 = q, 1 = k
                        wcol = p * 256 + qsel * 128
                        for cp in cps:             # 1024-col output tiles
                            ps = psA.tile([128, 1024], f32, tag="A")
                            for half in range(2):
                                t0 = cp * 1024 + half * 512
                                for c in range(CT):
                                    nc.tensor.matmul(
                                        ps[:, half * 512:half * 512 + 512],
                                        lhsT=wa[c][:, wcol:wcol + 128],
                                        rhs=xT[c][:, t0:t0 + 512],
                                        start=(c == 0),
                                        stop=(c == CT - 1 and not has_bias))
                                if has_bias:
                                    nc.tensor.matmul(
                                        ps[:, half * 512:half * 512 + 512],
                                        lhsT=wb[0:1, wcol:wcol + 128],
                                        rhs=ox[0:1, t0:t0 + 512],
                                        start=False, stop=True)
                            if qsel == 0:
                                nc.vector.tensor_copy(
                                    qw[p][:, cp * 1024:(cp + 1) * 1024],
                                    ps[:])
                            else:
                                for u in range(8):
                                    nc.vector.tensor_copy(
                                        kd[p][8 * cp + u][:],
                                        ps[:, u * 128:(u + 1) * 128])

            def emit_v(tp):                        # t-tile pair {2tp, 2tp+1}
                with nc.named_scope("qkv_v"):
                    ps = psA.tile([128, 1024], f32, tag="A")
                    for half in range(2):
                        t = 2 * tp + half
                        o = half * 512
                        for c in range(CT):
                            nc.tensor.matmul(
                                ps[:, o:o + 384],
                                lhsT=xT[c][:, t * 128:(t + 1) * 128],
                                rhs=wa[c][:, 768:1152],
                                start=(c == 0),
                                stop=(c == CT - 1 and not has_bias))
                        if has_bias:
                            nc.tensor.matmul(
                                ps[:, o:o + 384],
                                lhsT=ox[0:1, t * 128:(t + 1) * 128],
                                rhs=wb[0:1, 768:1152],
                                start=False, stop=True)
                    for half in range(2):
                        t = 2 * tp + half
                        o = half * 512
                        for h in range(HPC):
                            dst = 0 if h % 2 == 0 else 64
                            nc.vector.tensor_copy(
                                vo[t][h][:, dst:dst + 64],
                                ps[:, o + h * 64:o + h * 64 + 64])

            # ---- attention unit (pair p, q chunk j), split head/rest ----
            def attn_head(p, j):
                nblk = 4 * j + 4
                st = {"p": p, "j": j, "nblk": nblk,
                      "Ps": [None] * nblk, "ms": [None] * nblk}

                def emit_S(i):
                    m = i - 4 * j
                    lo = 128 * m if m >= 0 else 0
                    with nc.named_scope(f"attn{p}_{j}"):
                        sp = psA.tile([128, 1024], f32, tag="A")
                        for ab in range(2):
                            nc.tensor.matmul(
                                sp[:, ab * 512 + lo:(ab + 1) * 512],
                                lhsT=kd[p][i][ab * 64:(ab + 1) * 64, :],
                                rhs=qw[p][ab * 64:(ab + 1) * 64,
                                          j * 512 + lo:(j + 1) * 512],
                                start=True, stop=True)
                        P = pP.tile([128, 1024], bf16, tag="P")
                        if lo:
                            sp3 = sp[:].rearrange(
                                "p (h w) -> p h w", h=2)[:, :, lo:512]
                            P3 = P[:].rearrange(
                                "p (h w) -> p h w", h=2)[:, :, lo:512]
                            nc.scalar.activation(P3, sp3, Exp)
                        else:
                            nc.scalar.activation(P[:], sp[:], Exp)
                        st["Ps"][i], st["ms"][i] = P, max(m, 0)

                st["emit_S"] = emit_S
                for i in range(min(NHEAD, nblk)):
                    emit_S(i)
                return st

            def attn_rest(st, hook=None):
                p, j, nblk = st["p"], st["j"], st["nblk"]
                Ps, ms, emit_S = st["Ps"], st["ms"], st["emit_S"]
                with nc.named_scope(f"attn{p}_{j}"):
                    pv = psOD.tile([128, 1024], f32, tag="OD")

                    def emit_PV(i):
                        m = ms[i]
                        lo = 128 * m
                        P = Ps[i]
                        if m > 0 or i == 4 * j:
                            sl = P[:].rearrange(
                                "p (h w) -> p h w", h=2)[:, :, lo:lo + 128]
                            tri3 = tri_b[:].unsqueeze(1).broadcast_to(
                                [128, 2, 128])
                            nc.gpsimd.tensor_mul(sl, sl, tri3)
                        first, last = (i == 0), (i == nblk - 1)
                        # cols 0:512 <- [v_A|1]^T P_A = [O_A; den_A]
                        # cols 512:1024 <- [1|v_B]^T P_B = [den_B; O_B]
                        nc.tensor.matmul(
                            pv[:, lo:512], lhsT=vo[i][2 * p][:],
                            rhs=P[:, lo:512], start=first, stop=last)
                        nc.tensor.matmul(
                            pv[:, 512 + lo:1024], lhsT=vo[i][2 * p + 1][:],
                            rhs=P[:, 512 + lo:1024], start=first, stop=last)

                    # software pipeline; the next unit's first S blocks are
                    # emitted (via hook) before this unit's last PVs so the
                    # exp stream continues through the PV drain.
                    for i in range(NHEAD, nblk):
                        emit_S(i)
                        emit_PV(i - NHEAD)
                    if hook:
                        hook()
                    for i in range(max(0, nblk - NHEAD), nblk):
                        emit_PV(i)

                    # normalize: evacuate PV to SBUF (frees PSUM), then the
                    # reciprocal dance; multiplies on gpsimd.
                    sb = pst.tile([128, 1024], f32, tag="sb")
                    nc.vector.tensor_copy(sb[:], pv[:])
                    rB = pst.tile([64, 512], f32, tag="rB")
                    nc.vector.reciprocal_approx_fast(
                        rB[:], sb[0:64, 512:1024])
                    s2 = pst.tile([128, 512], f32, tag="rc")
                    nc.sync.dma_start(s2[0:64, :], sb[64:128, 0:512])
                    nc.sync.dma_start(s2[64:128, :], rB[:])
                    s3 = pst.tile([64, 512], f32, tag="s3")
                    nc.vector.reciprocal_approx_fast(s3[:], s2[0:64, :])
                    nc.gpsimd.tensor_mul(
                        yt[p][0:64, j * 512:(j + 1) * 512],
                        sb[0:64, 0:512], s3[:])
                    nc.gpsimd.tensor_mul(
                        yt[p][64:128, j * 512:(j + 1) * 512],
                        sb[64:128, 512:1024], s2[64:128, :])

            # ---- output projection of one t-tile ----
            def emit_proj_t(t):
                with nc.named_scope("proj"):
                    ps = psA.tile([128, 1024], f32, tag="A")
                    for n0, n1 in ((0, 512), (512, 768)):
                        for kk in range(NP):
                            nc.tensor.matmul(
                                ps[:, n0:n1],
                                lhsT=yt[kk][:, t * 128:(t + 1) * 128],
                                rhs=wp[kk][:, n0:n1],
                                start=(kk == 0), stop=(kk == NP - 1))
                    ob = pout.tile([128, C], f32, tag="o")
                    nc.vector.tensor_copy(ob[:], ps[:, 0:C])
                    nc.sync.dma_start(
                        out_ap[t * 128:(t + 1) * 128, :], ob[:])

            # ---- emission schedule ----
            # Unit order puts pair-0 chunks early (only qk0 needed) so the
            # scalar engine has exp work while the PE finishes qkv; proj of
            # chunk j is spread into later units once all pairs finished j.
            units = [(0, 0), (0, 1), (1, 0), (0, 2), (1, 1), (2, 0),
                     (0, 3), (1, 2), (2, 1), (1, 3), (2, 2), (2, 3)]
            # emissions to interleave before unit i (by unit index); a unit
            # (p, j) only needs its pair's q/k for column half j//2, so the
            # later halves are deferred into the attention stream.
            pre = {2: [lambda: emit_qk(1, (0,))],
                   3: [lambda: emit_qk(0, (1,))],
                   5: [lambda: emit_qk(2, (0,))],
                   7: [lambda: emit_qk(1, (1,))],
                   10: [lambda: emit_qk(2, (1,))]}
            # proj t-tiles to emit after unit i
            post = {6: [0, 1], 7: [2, 3], 8: [4, 5],
                    9: [6, 7], 10: [8, 9, 10, 11]}

            emit_qk(0, (0,))
            emit_v(0)
            emit_v(1)
            st = attn_head(*units[0])
            for tp in range(2, 8):
                emit_v(tp)
            for i in range(len(units)):
                for f in pre.get(i + 1, []):
                    f()
                nxt = {}

                def hook(i=i, nxt=nxt):
                    if i + 1 < len(units):
                        nxt["st"] = attn_head(*units[i + 1])
                attn_rest(st, hook=hook)
                st = nxt.get("st")
                for t in post.get(i, []):
                    emit_proj_t(t)
            for t in range(12, 16):
                emit_proj_t(t)

    nc.compile()
    return nc


def _prep_inputs(x, W_qkv, b_qkv, W_proj):
    """Per-core input maps (bf16 host arrays)."""
    import ml_dtypes
    bf = ml_dtypes.bfloat16
    sc = 1.0 / np.sqrt(D)
    tri = np.triu(np.ones((128, 128), dtype=np.float32)).astype(bf)
    in_maps = []
    for c in range(N_CORES):
        b, hh = c // 2, c % 2
        h0 = hh * 384                      # column offset of this half's heads
        # wa column order: q0 k0 q1 k1 q2 k2 (128 each) | v (384)
        pieces = []
        for p in range(NP):
            pieces.append(W_qkv[:, h0 + p * 128:h0 + (p + 1) * 128] * sc)
            pieces.append(W_qkv[:, 768 + h0 + p * 128:768 + h0 + (p + 1) * 128])
        pieces.append(W_qkv[:, 1536 + h0:1536 + h0 + 384])
        wa = np.ascontiguousarray(
            np.concatenate(pieces, axis=1)).astype(bf)
        m = {
            "xT": np.ascontiguousarray(x[b].T).astype(bf),
            "wa": wa,
            "wp": np.ascontiguousarray(W_proj[h0:h0 + 384, :]).astype(bf),
            "tri": tri,
        }
        if np.any(b_qkv):
            bp = []
            for p in range(NP):
                bp.append(b_qkv[h0 + p * 128:h0 + (p + 1) * 128] * sc)
                bp.append(b_qkv[768 + h0 + p * 128:768 + h0 + (p + 1) * 128])
            bp.append(b_qkv[1536 + h0:1536 + h0 + 384])
            m["ox"] = np.ones((1, T), dtype=np.float32).astype(bf)
            m["wb"] = np.concatenate(bp).reshape(1, 1152).astype(bf)
        in_maps.append(m)
    return in_maps


def _run(inputs, trace=False, tmpdir=None):
    from concourse.bass_utils import run_bass_kernel_spmd

    x = np.asarray(inputs["x"], dtype=np.float32)
    W_qkv = np.asarray(inputs["W_qkv"], dtype=np.float32)
    b_qkv = np.asarray(inputs["b_qkv"], dtype=np.float32)
    W_proj = np.asarray(inputs["W_proj"], dtype=np.float32)
    b_proj = np.asarray(inputs["b_proj"], dtype=np.float32)

    has_bias = bool(np.any(b_qkv))
    key = ("k", has_bias)
    if key not in _cache:
        _cache[key] = _build(has_bias)
    nc = _cache[key]

    in_maps = _prep_inputs(x, W_qkv, b_qkv, W_proj)
    res = run_bass_kernel_spmd(nc, in_maps, list(range(N_CORES)),
                               trace=trace, tmpdir=tmpdir)
    out = np.empty((B, T, C), dtype=np.float32)
    for b in range(B):
        out[b] = res.results[2 * b]["out"] + res.results[2 * b + 1]["out"]
    out += b_proj
    return out, res


def kernel(**inputs):
    out, _ = _run(inputs)
    return out


# revision 27
# speedup vs baseline: 1.0675x; 1.0675x over previous
"""Causal self-attention (B=4, T=2048, C=768, H=12) on 8 trn2 NeuronCores.

Sharding: core c -> batch b = c//2, head-half hh = c%2 (6 heads per core).
Each core computes, for its (b, 6 heads): qkv projection, causal attention,
and a partial output projection (its heads' rows of W_proj). The host sums
the two partial outputs per batch and adds b_proj.

All matmul operands are bf16 (fp32 accumulation in PSUM). Layouts keep the
PE contracting over partitions everywhere and softmax needs no transposes:
  - q^T [d, T] wide per pair; k^T stored as dense [128,128] tiles per
    (pair, k-block) so the S weight loads take the fast-weight-load path
  - S^T [tk, tq] blocks: lhsT = k tile, rhs = q^T chunk; two heads of a
    pair run concurrently via row groups (base partitions 0/64, K=64)
  - exp on ACT straight out of PSUM ([128, 1024] pair tiles, causal-skipped,
    diagonal strips merged into one 3D-AP activation)
  - PV: one dense [v_h | ones] (head A) / [ones | v_h] (head B) stationary
    per (t, head) -> a single full-width matmul per head per block computes
    both O^T and the softmax denominator replicated on the opposite 64
    partitions; normalize = reciprocal + partition-swap DMA + multiplies
    (multiplies on gpsimd, from an SBUF copy that frees PSUM early)
  - attention units are software-pipelined across unit boundaries (the next
    unit's first S blocks are emitted before the current unit's PV tail) so
    the scalar engine's exp stream never waits for the PV drain
  - proj is emitted per t-tile, spread into later units' slack; output DMA
    streams per tile
"""

import numpy as np

B, T, C = 4, 2048, 768
H = 12
D = C // H          # 64
HPC = 6             # heads per core
NP = 3              # head pairs per core
N_CORES = 8
TK = T // 128       # 16 t tiles
NCH = T // 512      # 4 q chunks
CT = C // 128       # 6 contraction tiles
NHEAD = 3           # S blocks emitted ahead at unit boundaries

_cache = {}


def _build(has_bias):
    import concourse.tile as tile
    from concourse import bacc, mybir

    dt = mybir.dt
    f32 = dt.float32
    bf16 = dt.bfloat16
    Exp = mybir.ActivationFunctionType.Exp

    nc = bacc.Bacc("TRN2", target_bir_lowering=False, debug=False,
                   num_devices=N_CORES)

    xT_ap = nc.dram_tensor("xT", [C, T], bf16, kind="ExternalInput").ap()
    wa_ap = nc.dram_tensor("wa", [C, 1152], bf16, kind="ExternalInput").ap()
    wp_ap = nc.dram_tensor("wp", [HPC * D, C], bf16, kind="ExternalInput").ap()
    tri_ap = nc.dram_tensor("tri", [128, 128], bf16, kind="ExternalInput").ap()
    if has_bias:
        ox_ap = nc.dram_tensor("ox", [1, T], bf16, kind="ExternalInput").ap()
        wb_ap = nc.dram_tensor("wb", [1, 1152], bf16, kind="ExternalInput").ap()
    out_ap = nc.dram_tensor("out", [T, C], f32, kind="ExternalOutput").ap()

    with tile.TileContext(nc) as tc:
        with tc.tile_pool(name="pers", bufs=1) as pers, \
             tc.tile_pool(name="pP", bufs=6) as pP, \
             tc.tile_pool(name="pst", bufs=2) as pst, \
             tc.tile_pool(name="pout", bufs=2) as pout, \
             tc.tile_pool(name="psA", bufs=2, space="PSUM") as psA, \
             tc.tile_pool(name="psOD", bufs=2, space="PSUM") as psOD:

            # ---- persistent SBUF tensors + staged input DMA ----
            # wa columns (host layout): q0 k0 q1 k1 q2 k2 (128 each) | v (384)
            xT = [pers.tile([128, T], bf16, tag=f"x{i}", name=f"x{i}")
                  for i in range(CT)]
            wa = [pers.tile([128, 1152], bf16, tag=f"w{i}", name=f"w{i}")
                  for i in range(CT)]
            # priority order: pair0 q/k weights interleaved with x's first
            # quarter, then the rest of x, v weights, pair1/2 q/k weights.
            for i in range(CT):
                nc.sync.dma_start(wa[i][:, 0:256],
                                  wa_ap[i * 128:(i + 1) * 128, 0:256])
                nc.sync.dma_start(xT[i][:, 0:512],
                                  xT_ap[i * 128:(i + 1) * 128, 0:512])
            for q in range(1, 4):
                for i in range(CT):
                    nc.sync.dma_start(xT[i][:, q * 512:(q + 1) * 512],
                                      xT_ap[i * 128:(i + 1) * 128,
                                            q * 512:(q + 1) * 512])
            tri_b = pers.tile([128, 128], bf16, tag="trib")
            nc.sync.dma_start(tri_b[:], tri_ap)
            for i in range(CT):
                nc.sync.dma_start(wa[i][:, 768:1152],
                                  wa_ap[i * 128:(i + 1) * 128, 768:1152])
            for i in range(CT):
                nc.sync.dma_start(wa[i][:, 256:768],
                                  wa_ap[i * 128:(i + 1) * 128, 256:768])
            wp = [pers.tile([128, C], bf16, tag=f"p{i}", name=f"wp{i}")
                  for i in range(NP)]
            for i in range(NP):
                nc.sync.dma_start(wp[i][:], wp_ap[i * 128:(i + 1) * 128, :])
            if has_bias:
                ox = pers.tile([1, T], bf16, tag="ox")
                nc.sync.dma_start(ox[:], ox_ap)
                wb = pers.tile([1, 1152], bf16, tag="wb")
                nc.sync.dma_start(wb[:], wb_ap)
            ones_f = pers.tile([128, 64], f32, tag="onesf")
            nc.vector.memset(ones_f[:], 1.0)
            ones_b = pers.tile([128, 64], bf16, tag="onesb")
            nc.vector.tensor_copy(ones_b[:], ones_f[:])

            # q^T wide per pair; k as dense [128,128] tiles per (pair, i)
            qw = [pers.tile([128, T], bf16, tag=f"q{p}", name=f"q{p}")
                  for p in range(NP)]
            kd = [[pers.tile([128, 128], bf16, tag=f"k{p}_{i}",
                             name=f"k{p}_{i}")
                   for i in range(TK)] for p in range(NP)]
            # [v_h | ones] (even h) / [ones | v_h] (odd h) stationaries
            vo = [[pers.tile([128, 128], bf16, tag=f"v{t}_{h}",
                             name=f"v{t}_{h}")
                   for h in range(HPC)] for t in range(TK)]
            # ones halves via gpsimd (idle at startup); DMAing them would
            # clog the sync engine's DMA-trigger queue for ~90us.
            for t in range(TK):
                for h in range(HPC):
                    half = 64 if h % 2 == 0 else 0
                    nc.gpsimd.tensor_copy(vo[t][h][:, half:half + 64],
                                          ones_b[:])
            yt = [pers.tile([128, T], bf16, tag=f"y{p}", name=f"y{p}")
                  for p in range(NP)]

            # ---- qkv projection pieces (per 1024-col half of T) ----
            def emit_qk(p, cps=(0, 1)):
                with nc.named_scope(f"qk{p}"):
                    for qsel in range(2):          # 0 = q, 1 = k
                        wcol = p * 256 + qsel * 128
                        for cp in cps:             # 1024-col output tiles
                            ps = psA.tile([128, 1024], f32, tag="A")
                            for half in range(2):
                                t0 = cp * 1024 + half * 512
                                for c in range(CT):
                                    nc.tensor.matmul(
                                        ps[:, half * 512:half * 512 + 512],
                                        lhsT=wa[c][:, wcol:wcol + 128],
                                        rhs=xT[c][:, t0:t0 + 512],
                                        start=(c == 0),
                                        stop=(c == CT - 1 and not has_bias))
                                if has_bias:
                                    nc.tensor.matmul(
                                        ps[:, half * 512:half * 512 + 512],
                                        lhsT=wb[0:1, wcol:wcol + 128],
                                        rhs=ox[0:1, t0:t0 + 512],
                                        start=False, stop=True)
                            if qsel == 0:
                                nc.vector.tensor_copy(
                                    qw[p][:, cp * 1024:(cp + 1) * 1024],
                                    ps[:])
                            else:
                                for u in range(8):
                                    nc.vector.tensor_copy(
                                        kd[p][8 * cp + u][:],
                                        ps[:, u * 128:(u + 1) * 128])

            def emit_v(tp):                        # t-tile pair {2tp, 2tp+1}
                with nc.named_scope("qkv_v"):
                    ps = psA.tile([128, 1024], f32, tag="A")
                    for half in range(2):
                        t = 2 * tp + half
                        o = half * 512
                        for c in range(CT):
                            nc.tensor.matmul(
                                ps[:, o:o + 384],
                                lhsT=xT[c][:, t * 128:(t + 1) * 128],
                                rhs=wa[c][:, 768:1152],
                                start=(c == 0),
                                stop=(c == CT - 1 and not has_bias))
                        if has_bias:
                            nc.tensor.matmul(
                                ps[:, o:o + 384],
                                lhsT=ox[0:1, t * 128:(t + 1) * 128],
                                rhs=wb[0:1, 768:1152],
                                start=False, stop=True)
                    for half in range(2):
                        t = 2 * tp + half
                        o = half * 512
                        for h in range(HPC):
                            dst = 0 if h % 2 == 0 else 64
                            nc.vector.tensor_copy(
                                vo[t][h][:, dst:dst + 64],
                                ps[:, o + h * 64:o + h * 64 + 64])

            # ---- attention unit (pair p, q chunk j), split head/rest ----
            def attn_head(p, j):
                nblk = 4 * j + 4
                st = {"p": p, "j": j, "nblk": nblk,
                      "Ps": [None] * nblk, "ms": [None] * nblk}

                def emit_S(i):
                    m = i - 4 * j
                    lo = 128 * m if m >= 0 else 0
                    with nc.named_scope(f"attn{p}_{j}"):
                        sp = psA.tile([128, 1024], f32, tag="A")
                        for ab in range(2):
                            nc.tensor.matmul(
                                sp[:, ab * 512 + lo:(ab + 1) * 512],
                                lhsT=kd[p][i][ab * 64:(ab + 1) * 64, :],
                                rhs=qw[p][ab * 64:(ab + 1) * 64,
                                          j * 512 + lo:(j + 1) * 512],
                                start=True, stop=True)
                        P = pP.tile([128, 1024], bf16, tag="P")
                        if lo:
                            sp3 = sp[:].rearrange(
                                "p (h w) -> p h w", h=2)[:, :, lo:512]
                            P3 = P[:].rearrange(
                                "p (h w) -> p h w", h=2)[:, :, lo:512]
                            nc.scalar.activation(P3, sp3, Exp)
                        else:
                            nc.scalar.activation(P[:], sp[:], Exp)
                        st["Ps"][i], st["ms"][i] = P, max(m, 0)

                st["emit_S"] = emit_S
                for i in range(min(NHEAD, nblk)):
                    emit_S(i)
                return st

            def attn_rest(st, hook=None):
                p, j, nblk = st["p"], st["j"], st["nblk"]
                Ps, ms, emit_S = st["Ps"], st["ms"], st["emit_S"]
                with nc.named_scope(f"attn{p}_{j}"):
                    pv = psOD.tile([128, 1024], f32, tag="OD")

                    def emit_PV(i):
                        m = ms[i]
                        lo = 128 * m
                        P = Ps[i]
                        if m > 0 or i == 4 * j:
                            sl = P[:].rearrange(
                                "p (h w) -> p h w", h=2)[:, :, lo:lo + 128]
                            tri3 = tri_b[:].unsqueeze(1).broadcast_to(
                                [128, 2, 128])
                            nc.gpsimd.tensor_mul(sl, sl, tri3)
                        first, last = (i == 0), (i == nblk - 1)
                        # cols 0:512 <- [v_A|1]^T P_A = [O_A; den_A]
                        # cols 512:1024 <- [1|v_B]^T P_B = [den_B; O_B]
                        nc.tensor.matmul(
                            pv[:, lo:512], lhsT=vo[i][2 * p][:],
                            rhs=P[:, lo:512], start=first, stop=last)
                        nc.tensor.matmul(
                            pv[:, 512 + lo:1024], lhsT=vo[i][2 * p + 1][:],
                            rhs=P[:, 512 + lo:1024], start=first, stop=last)

                    # software pipeline; the next unit's first S blocks are
                    # emitted (via hook) before this unit's last PVs so the
                    # exp stream continues through the PV drain.
                    for i in range(NHEAD, nblk):
                        emit_S(i)
                        emit_PV(i - NHEAD)
                    if hook:
                        hook()
                    for i in range(max(0, nblk - NHEAD), nblk):
                        emit_PV(i)

                    # normalize: evacuate PV to SBUF (frees PSUM), then the
                    # reciprocal dance; multiplies on gpsimd.
                    sb = pst.tile([128, 1024], f32, tag="sb")
                    nc.vector.tensor_copy(sb[:], pv[:])
                    rB = pst.tile([64, 512], f32, tag="rB")
                    nc.vector.reciprocal_approx_fast(
                        rB[:], sb[0:64, 512:1024])
                    s2 = pst.tile([128, 512], f32, tag="rc")
                    nc.sync.dma_start(s2[0:64, :], sb[64:128, 0:512])
                    nc.sync.dma_start(s2[64:128, :], rB[:])
                    s3 = pst.tile([64, 512], f32, tag="s3")
                    nc.vector.reciprocal_approx_fast(s3[:], s2[0:64, :])
                    nc.gpsimd.tensor_mul(
                        yt[p][0:64, j * 512:(j + 1) * 512],
                        sb[0:64, 0:512], s3[:])
                    nc.gpsimd.tensor_mul(
                        yt[p][64:128, j * 512:(j + 1) * 512],
                        sb[64:128, 512:1024], s2[64:128, :])

            # ---- output projection of one t-tile ----
            def emit_proj_t(t):
                with nc.named_scope("proj"):
                    ps = psA.tile([128, 1024], f32, tag="A")
                    for n0, n1 in ((0, 512), (512, 768)):
                        for kk in range(NP):
                            nc.tensor.matmul(
                                ps[:, n0:n1],
                                lhsT=yt[kk][:, t * 128:(t + 1) * 128],
                                rhs=wp[kk][:, n0:n1],
                                start=(kk == 0), stop=(kk == NP - 1))
                    ob = pout.tile([128, C], f32, tag="o")
                    nc.vector.tensor_copy(ob[:], ps[:, 0:C])
                    nc.sync.dma_start(
                        out_ap[t * 128:(t + 1) * 128, :], ob[:])

            # ---- emission schedule ----
            # Unit order puts pair-0 chunks early (only qk0 needed) so the
            # scalar engine has exp work while the PE finishes qkv; proj of
            # chunk j is spread into later units once all pairs finished j.
            units = [(0, 0), (0, 1), (1, 0), (0, 2), (1, 1), (2, 0),
                     (0, 3), (1, 2), (2, 1), (1, 3), (2, 2), (2, 3)]
            # emissions to interleave before unit i (by unit index); a unit
            # (p, j) only needs its pair's q/k for column half j//2, so the
            # later halves are deferred into the attention stream.
            pre = {2: [lambda: emit_qk(1, (0,))],
                   3: [lambda: emit_qk(0, (1,))],
                   5: [lambda: emit_qk(2, (0,))],
                   7: [lambda: emit_qk(1, (1,))],
                   10: [lambda: emit_qk(2, (1,))]}
            # proj t-tiles to emit after unit i
            post = {6: [0, 1], 7: [2, 3], 8: [4, 5],
                    9: [6, 7], 10: [8, 9, 10, 11]}

            emit_qk(0, (0,))
            emit_v(0)
            emit_v(1)
            st = attn_head(*units[0])
            for tp in range(2, 8):
                emit_v(tp)
            for i in range(len(units)):
                for f in pre.get(i + 1, []):
                    f()
                nxt = {}

                def hook(i=i, nxt=nxt):
                    if i + 1 < len(units):
                        nxt["st"] = attn_head(*units[i + 1])
                attn_rest(st, hook=hook)
                st = nxt.get("st")
                for t in post.get(i, []):
                    emit_proj_t(t)
            for t in range(12, 16):
                emit_proj_t(t)

    nc.compile()
    return nc


def _prep_inputs(x, W_qkv, b_qkv, W_proj):
    """Per-core input maps (bf16 host arrays)."""
    import ml_dtypes
    bf = ml_dtypes.bfloat16
    sc = 1.0 / np.sqrt(D)
    tri = np.triu(np.ones((128, 128), dtype=np.float32)).astype(bf)
    in_maps = []
    for c in range(N_CORES):
        b, hh = c // 2, c % 2
        h0 = hh * 384                      # column offset of this half's heads
        # wa column order: q0 k0 q1 k1 q2 k2 (128 each) | v (384)
        pieces = []
        for p in range(NP):
            pieces.append(W_qkv[:, h0 + p * 128:h0 + (p + 1) * 128] * sc)
            pieces.append(W_qkv[:, 768 + h0 + p * 128:768 + h0 + (p + 1) * 128])
        pieces.append(W_qkv[:, 1536 + h0:1536 + h0 + 384])
        wa = np.ascontiguousarray(
            np.concatenate(pieces, axis=1)).astype(bf)
        m = {
            "xT": np.ascontiguousarray(x[b].T).astype(bf),
            "wa": wa,
            "wp": np.ascontiguousarray(W_proj[h0:h0 + 384, :]).astype(bf),
            "tri": tri,
        }
        if np.any(b_qkv):
            bp = []
            for p in range(NP):
                bp.append(b_qkv[h0 + p * 128:h0 + (p + 1) * 128] * sc)
                bp.append(b_qkv[768 + h0 + p * 128:768 + h0 + (p + 1) * 128])
            bp.append(b_qkv[1536 + h0:1536 + h0 + 384])
            m["ox"] = np.ones((1, T), dtype=np.float32).astype(bf)
            m["wb"] = np.concatenate(bp).reshape(1, 1152).astype(bf)
        in_maps.append(m)
    return in_maps


def _run(inputs, trace=False, tmpdir=None):
    from concourse.bass_utils import run_bass_kernel_spmd

    x = np.asarray(inputs["x"], dtype=np.float32)
    W_qkv = np.asarray(inputs["W_qkv"], dtype=np.float32)
    b_qkv = np.asarray(inputs["b_qkv"], dtype=np.float32)
    W_proj = np.asarray(inputs["W_proj"], dtype=np.float32)
    b_proj = np.asarray(inputs["b_proj"], dtype=np.float32)

    has_bias = bool(np.any(b_qkv))
    key = ("k", has_bias)
    if key not in _cache:
        _cache[key] = _build(has_bias)
    nc = _cache[key]

    in_maps = _prep_inputs(x, W_qkv, b_qkv, W_proj)
    res = run_bass_kernel_spmd(nc, in_maps, list(range(N_CORES)),
                               trace=trace, tmpdir=tmpdir)
    out = np.empty((B, T, C), dtype=np.float32)
    for b in range(B):
        out[b] = res.results[2 * b]["out"] + res.results[2 * b + 1]["out"]
    out += b_proj
    return out, res


def kernel(**inputs):
    out, _ = _run(inputs)
    return out
